# revision 55
# baseline (speedup 1.0000x reference)
"""Trainium2 Bass kernel for multi-head quadratic spatial attention.

Problem: q,k,v [b=8, heads=8, h=32, w=32, d=64] fp32; full attention over
the 1024-position spatial grid independently per (b, head); output
[b, h, w, heads*d].

Sharding: data-parallel over batch — core c handles b=c (8 heads of
[1024, 64] attention per core), no cross-core communication.

Per-core pipeline (heads in PAIRS; matmuls bf16, fp32 PSUM accumulation).
The PE streams columns serially (no real tile concurrency on this
toolchain), so the design minimizes streamed columns and keeps the HAM
clock gate at K=8/8 (any sustained PE-idle re-throttles to 1.2 GHz):
  - p-major seq tiling (seq = p*8 + t); 4D casting DMAs per (tensor,
    pair); [128,128] PE transposes build the d-major qt/kt tiles, whose
    PSUM staging is evacuated by ScalarE copies
  - mm1: per step, head A's block jb and head B's block jb-1 (one-step
    software shift) -> St [128, 1024] fp32 tiles from a SHARED 3-deep
    PSUM ring (6 banks).  The A,B,A,B allocation cadence lands every WAR
    on an exp >= 1 full step old: the exp latency chain never gates the
    PE, and every cross-engine wait points backward (deadlock-free by
    construction)
  - exp split across engines: head A exact Exp on ScalarE, head B on
    VectorE via the Schraudolph bit-trick (fused tensor_scalar mult+add
    -> int16 == bf16 exp approx, ~3% weight error that largely cancels
    in the softmax average); both heads' St tiles release concurrently
  - mm2 FLIPPED: stationary = Pt j-block [128,128] (full-column bf16 ->
    FWL, ldw hides under the stream; HW-measured ~29-54 ns/matmul),
    moving = [V(jb) | ones] [128, 65] -> output lands i-MAJOR [128, 65]
    accumulated over j-blocks, so no transpose-back epilogue exists and
    the ones column rides along as the softmax denominator.  Runs as 8
    half-unit bursts per pair (one per step) to keep PE density high
  - epilogue: VectorE reciprocal of the denominator column + broadcast
    tensor_mul normalize into fp32 ostage; per-quad stores on the sync
    HWDGE ring
  - HAM food: one ~107ns filler matmul per step into the next stA tile
    (the real mm1's start=True clears has_written, so it is free) plus a
    short cold-start warm-up; without it the activity monitor
    duty-throttles the PE clock mid-kernel
"""

from contextlib import ExitStack

import numpy as np

F32 = None
BF16 = None
I16 = None

_cache = {}

# Schraudolph exp in bf16 bit-space: bf16_bits(exp(s*x)) ~= round(x*A + B)
# A = s * 2^7/ln2, B = 2^7*(127 - sigma), sigma = 0.0430 balances the
# piecewise-linear 2^frac error (max rel err ~3%, mostly cancelling in the
# softmax average).
SCALE = 64.0 ** -0.5
SCHRAUD_A = SCALE * 128.0 / float(np.log(2.0))
SCHRAUD_B = 128.0 * (127.0 - 0.0430)

N_WARM = 6  # dummy matmuls to flip the PE HAM clock gate before real work

# (jb, head-in-pair) St tiles exp'd on VectorE via the Schraudolph bit-trick
# instead of ScalarE's exact exp. Head B's tiles all go to VectorE: exp-A
# (ScalarE, ~1.15us) and exp-B (VectorE, ~1.19us) then run CONCURRENTLY, so
# stA/stB release near-simultaneously and the scheduler can bake the mm1
# quartet as A,B,A,B (adjacent disjoint row groups -> concurrent on the PE).
SCHRAUD_TILES = frozenset((jb, 1) for jb in range(8))


def _imports():
    global F32, BF16, I16
    import concourse.bass as bass
    import concourse.tile as tile
    from concourse import mybir
    from concourse.masks import make_identity

    F32 = mybir.dt.float32
    BF16 = mybir.dt.bfloat16
    I16 = mybir.dt.int16
    return bass, tile, mybir, make_identity


def _split_multi_waits(nc, mybir):
    """Walrus in this container supports only ONE sync-wait per instruction.
    Hoist extra waits onto same-engine InstNoOp's inserted just before."""
    ctr = 0
    for f in nc.m.functions:
        for bb in f.blocks:
            insts = bb.instructions
            if not any(
                i.sync_info and i.sync_info.on_wait and len(i.sync_info.on_wait) > 1
                for i in insts
            ):
                continue
            out = []
            for inst in insts:
                si = inst.sync_info
                waits = list(si.on_wait) if si and si.on_wait else []
                if len(waits) > 1:
                    for w in waits[:-1]:
                        ctr += 1
                        nop = mybir.InstNoOp(
                            name=f"I-wsplit-{ctr}",
                            engine=inst.engine,
                            ins=[],
                            outs=[],
                            sync_info=mybir.SyncInfo(on_wait=[w], on_update=[]),
                        )
                        nc.register_instruction(nop)
                        out.append(nop)
                    si.on_wait = waits[-1:]
                out.append(inst)
            bb.instructions = out


def _build_nc(heads=8, seq=1024, d=64):
    bass, tile, mybir, make_identity = _imports()
    assert heads % 2 == 0 and seq == 1024 and d == 64
    nt = seq // 128          # 8 blocks of 128 positions
    nh = seq // 512          # 2 i-halves of 512
    dv = d + 1
    TS_MULT = mybir.AluOpType.mult
    TS_ADD = mybir.AluOpType.add

    nc = bass.Bass(trn_type="TRN2", target_bir_lowering=False)
    q_d = nc.dram_tensor("q", [heads, seq, d], F32, kind="ExternalInput")
    k_d = nc.dram_tensor("k", [heads, seq, d], F32, kind="ExternalInput")
    v_d = nc.dram_tensor("v", [heads, seq, d], F32, kind="ExternalInput")
    o_d = nc.dram_tensor("out", [seq, heads * d], F32, kind="ExternalOutput")

    # p-major: seq = p*nt + t; per-(p, t) HBM runs are 256B contiguous
    q_ap = q_d[:].rearrange("n (p t) d -> n p t d", p=128)
    k_ap = k_d[:].rearrange("n (p t) d -> n p t d", p=128)
    v_ap = v_d[:].rearrange("n (p t) d -> n p t d", p=128)
    o_ap = o_d[:].rearrange("(p t) c -> p t c", p=128)

    with tile.TileContext(nc) as tc, ExitStack() as ctx:
        consts = ctx.enter_context(tc.tile_pool(name="consts", bufs=1))
        nat = ctx.enter_context(tc.tile_pool(name="nat", bufs=2))
        dmaj = ctx.enter_context(tc.tile_pool(name="dmaj", bufs=2))
        ptp = ctx.enter_context(tc.tile_pool(name="ptp", bufs=36))
        outp = ctx.enter_context(tc.tile_pool(name="outp", bufs=3))
        small = ctx.enter_context(tc.tile_pool(name="small", bufs=4))

        # PSUM banks: st 2x2 (0-3) + oacc/ob/warm 2x1 (4-5) + tp 2x1 (6-7)
        # PSUM (8 banks): stA ring 2x2 (4) + stB 1x2 (2) + shared scratch
        # ring 2x1 (2) carrying input-transpose staging, warm filler tiles
        # and the mm2 i-major accumulators (lifetimes interleave cleanly:
        # loads sit at pair boundaries, mm2 units mid-pair).
        st_ps = ctx.enter_context(tc.tile_pool(name="st_ps", bufs=2, space="PSUM"))
        scr_ps = ctx.enter_context(tc.tile_pool(name="scr_ps", bufs=2, space="PSUM"))

        ident_bf = consts.tile([128, 128], BF16)
        make_identity(nc, ident_bf[:])

        # Warm-up / filler matmuls keep the PE HAM clock gate at 2.4 GHz:
        # an idle (or transpose-only) stretch > ~3.4us re-throttles the PE
        # clock to 1.2 GHz for the next several microseconds. wsrc is
        # memset-ready within ~200ns of kernel start. N=512 streams give
        # ~213ns of HAM-counted busy per filler instruction.
        wsrc = consts.tile([128, 512], BF16)
        nc.vector.memset(wsrc[:], 0.25)

        def pe_filler(n):
            # fresh tile per burst: fillers WAR-chain only onto transient
            # transpose tiles, never onto live oacc accumulators
            t = scr_ps.tile([128, 512], F32, tag="scr", name="warm")
            for _ in range(n):
                nc.tensor.matmul(
                    t[:], wsrc[:, 0:128], wsrc[:], start=True, stop=True
                )

        pe_filler(N_WARM)

        def load_and_transpose(pair):
            """DMA pair inputs (bf16 cast, one 4D DMA per tensor) and build
            packed d-major tiles: head A on partitions 0:64, head B on
            64:128 (one [128,128] PE transpose per block)."""
            st8 = {"heads": (2 * pair, 2 * pair + 1), "v": None, "pts": {},
                   "oacc": {}, "ostage": {}}
            # pair-interleaved natural tiles: [..., 2, d] with head A at
            # index 0 and head B at 1, so one [128, 128] PE transpose of a
            # block yields A's d-rows on partitions 0:64 and B's on 64:128.
            qp = nat.tile([128, nt, 2, d], BF16, tag="qp")
            kp = nat.tile([128, nt, 2, d], BF16, tag="kp")
            hh = nt // 2
            if pair == 0:
                # halved loads, first halves of BOTH heads first, so the
                # first transpose group can start after ~2 trigger slots
                for lo, hi in ((0, hh), (hh, nt)):
                    for src_ap, dst in ((q_ap, qp), (k_ap, kp)):
                        for idx, n in enumerate(st8["heads"]):
                            nc.gpsimd.dma_start(
                                out=dst[:, lo:hi, idx, :], in_=src_ap[n, :, lo:hi]
                            )
            else:
                for idx, n in enumerate(st8["heads"]):
                    nc.gpsimd.dma_start(out=qp[:, :, idx, :], in_=q_ap[n])
                    nc.gpsimd.dma_start(out=kp[:, :, idx, :], in_=k_ap[n])
            vp = nat.tile([128, nt, 2, dv], BF16, tag="vp")
            # ones columns for the softmax-denominator trick
            nc.vector.memset(vp[:, :, :, d : d + 1], 1.0)
            for idx, n in enumerate(st8["heads"]):
                nc.gpsimd.dma_start(out=vp[:, :, idx, 0:d], in_=v_ap[n])
            st8["v"] = vp
            qt = dmaj.tile([128, seq], BF16, tag="qt")
            kt = dmaj.tile([128, seq], BF16, tag="kt")
            for g in range(nt // 4):
                for src, dst in ((qp, qt), (kp, kt)):
                    # pad to 2KB so every scr-ring slot is one full bank
                    tp = scr_ps.tile([128, 1024], BF16, tag="scr")
                    for u in range(4):
                        t = g * 4 + u
                        nc.tensor.transpose(
                            tp[:, u * 128 : (u + 1) * 128],
                            src[:, t, :, :],
                            ident_bf[:],
                        )
                    # ScalarE evacuation: the DVE must NOT carry this -- a
                    # DVE-queue copy waiting on PE transposes behind a PE
                    # mm1-B that waits the (1-deep) stB WAR on the DVE
                    # closes a deadlock cycle.  ScalarE's PE waits (stA,
                    # 2-deep ring) are loose, so no cycle can form there.
                    nc.scalar.copy(
                        out=dst[:, g * 512 : (g + 1) * 512], in_=tp[:, 0:512]
                    )
                    if pair == 0:
                        # PE is otherwise DMA-bound here; keep the clock warm
                        pe_filler(3)
            st8["qt"], st8["kt"] = qt, kt
            return st8

        def _exp(s, jb, idx, st):
            """Evacuate one St tile: exact Exp on ScalarE, or the Schraudolph
            bit-trick on VectorE for tiles in SCHRAUD_TILES.  High priority:
            St must evacuate ASAP to release PSUM for the next mm1 quartet,
            ahead of same-engine epilogue work (ot copies / normalize)."""
            with tc.high_priority(offset=30):
                pt = ptp.tile([128, seq], BF16, name="pt", tag="pt")
                if (jb, idx) in SCHRAUD_TILES:
                    nc.vector.tensor_scalar(
                        out=pt[:].bitcast(I16),
                        in0=st[:],
                        scalar1=SCHRAUD_A,
                        scalar2=SCHRAUD_B,
                        op0=TS_MULT,
                        op1=TS_ADD,
                    )
                else:
                    nc.scalar.activation(
                        out=pt[:],
                        in_=st[:],
                        func=mybir.ActivationFunctionType.Exp,
                        scale=SCALE,
                    )
                s["pts"][(jb, idx)] = pt

        def mm1_exp(s, jb):
            """One software-pipelined mm1 step: head A's block jb together
            with head B's block jb-1 (B SHIFTED ONE STEP behind A).

            Head A contracts on PE rows 0:64 (row group h0), head B on
            64:128 (h64).  Emitted interleaved A(c),B(c): consecutive
            instructions target DISJOINT row groups, so the PE streams them
            CONCURRENTLY (~2x issue rate vs same-group runs).  The one-step
            B shift is what makes this robust: B(jb-1)'s PSUM slot was
            released by exp-B(jb-2), a full step ago, so whenever A(jb)
            becomes ready B is ready too and the scheduler bakes the
            quartet adjacently instead of splitting it around mm2 work."""
            qt, kt = s["qt"], s["kt"]
            # shared 3-deep ring (6 PSUM banks): the A,B,A,B allocation
            # cadence lands every stA WAR on an exp TWO steps old and every
            # stB WAR on the PREVIOUS step's ScalarE exp-A -- all mm1 gates
            # point backward with slack, which both dissolves the exp-A
            # latency chain and makes cross-engine deadlock impossible.
            stA = st_ps.tile([128, seq], F32, name="stA", tag="st", bufs=3)
            stB = None
            if jb > 0:
                stB = st_ps.tile([128, seq], F32, name="stB", tag="st", bufs=3)
            # ~107ns of HAM food per step: the PE runs ~0.1us under the
            # engine-paced step, and without it the activity monitor sees
            # enough idle to duty-throttle the clock (K=4/8) mid-kernel.
            # Writing into stA is free: the real mm1's start=True clears
            # has_written and overwrites.
            nc.tensor.matmul(
                stA[:, 0:256], wsrc[:, 0:128], wsrc[:, 0:256],
                start=True, stop=True,
            )
            # A's chunks FIRST and adjacent: exp-A(jb) gates the next step's
            # mm1-A via the St PSUM ring (the critical latency chain), so it
            # must start as early as possible.  B(jb-1)'s chunks follow; its
            # exp has a full step of slack.
            # INTERLEAVED A,B,A,B: adjacent disjoint row groups genuinely
            # stream CONCURRENTLY on this hardware (HW-measured dstart
            # 3-23ns with unchanged per-matmul durations) -- but only when
            # both tiles are long-ready, which the 3-deep ring guarantees.
            # Interleaved emission makes the scheduler bake every quartet
            # that way instead of ~half of them.
            for c in range(nh):
                nc.tensor.matmul(
                    stA[:, c * 512 : (c + 1) * 512],
                    kt[0:64, jb * 128 : (jb + 1) * 128],
                    qt[0:64, c * 512 : (c + 1) * 512],
                    start=True,
                    stop=True,
                )
                if stB is not None:
                    nc.tensor.matmul(
                        stB[:, c * 512 : (c + 1) * 512],
                        kt[64:128, (jb - 1) * 128 : jb * 128],
                        qt[64:128, c * 512 : (c + 1) * 512],
                        start=True,
                        stop=True,
                    )
            _exp(s, jb, 0, stA)
            if stB is not None:
                _exp(s, jb - 1, 1, stB)

        def mm1_tail(s):
            """Head B's last block (jb=nt-1), deferred by the one-step
            shift."""
            qt, kt = s["qt"], s["kt"]
            stB = st_ps.tile([128, seq], F32, name="stB", tag="st", bufs=3)
            for c in range(nh):
                nc.tensor.matmul(
                    stB[:, c * 512 : (c + 1) * 512],
                    kt[64:128, (nt - 1) * 128 : nt * 128],
                    qt[64:128, c * 512 : (c + 1) * 512],
                    start=True,
                    stop=True,
                )
            _exp(s, nt - 1, 1, stB)

        def mm2_unit(s, u):
            """FLIPPED mm2 for one (head, i-quad): stationary = Pt j-block
            [128, 128] (FWL-eligible: full 128 bf16 columns), moving =
            [V(jb) | ones] [128, 65].  The 128-col LDWEIGHTS hides entirely
            behind the previous 65-col stream (HW-measured ~29 ns/matmul
            sustained vs ~216 ns for the V-stationary orientation), and the
            output lands i-MAJOR [128 i, 65], so the whole transpose-back
            epilogue (ot copy + 4 PE transposes + ob stage) disappears.
            Accumulation is ib-outer/jb-inner: start=True clears has_written
            for the WHOLE bank, so each ib region must fully accumulate
            before the next region's start."""
            idx, quad, half = u // 4, (u // 2) % 2, u % 2
            n = s["heads"][idx]
            # [128, 4, 128] fp32 = exactly one PSUM bank; dv slice per ib.
            # Each (head, quad) runs as TWO half-units of 16 matmuls on
            # consecutive steps: one ~0.5us mm2 burst every step keeps the
            # PE dense enough that the HAM activity monitor stays at K=8/8.
            # ONLY the tile's very first matmul uses start=True: its
            # has_written clear covers the WHOLE bank, so later regions'
            # first writes overwrite via still-clear bits.  A per-region
            # start=True would re-clear the bank and can race the previous
            # region's in-flight drain columns (observed as a rare NaN).
            if half == 0:
                s["oacc"][(idx, quad)] = scr_ps.tile(
                    [128, 4, 128], F32, name="oacc", tag="scr"
                )
            oacc = s["oacc"][(idx, quad)]
            for k in range(2 * half, 2 * half + 2):
                ib = quad * 4 + k
                for jb in range(nt):
                    pt = s["pts"][(jb, idx)]
                    nc.tensor.matmul(
                        oacc[:, k, 0:dv],
                        pt[:, ib * 128 : (ib + 1) * 128],
                        s["v"][:, jb, idx, 0:dv],
                        start=(jb == 0 and k == 0),
                        stop=(jb == nt - 1 and k == 3),
                        # single bank-clear protocol: the sim's per-region
                        # group check doesn't model it; hw stop is a no-op
                        skip_group_check=True,
                    )
            if half == 0:
                return
            if idx not in s["ostage"]:
                s["ostage"][idx] = outp.tile(
                    [128, nt, d], F32, name="ostage", tag="ostage"
                )
            ostage = s["ostage"][idx]
            # out partition p of block ib <-> seq p*8 + ib: ostage[:, ib, :]
            rec = small.tile([128, 4], F32, tag="rec")
            nc.vector.reciprocal(out=rec[:], in_=oacc[:, :, d])
            # all-DVE normalize: ScalarE's queue must stay trivially
            # forward-progressing (only exp/copies waiting on earlier PE
            # work); an ACT normalize waiting on a future mm2 stop can
            # close a cross-engine ordering cycle with the mm1 WAR gates.
            nc.vector.tensor_mul(
                ostage[:, quad * 4 : (quad + 1) * 4, :],
                oacc[:, :, 0:d],
                rec[:, :, None].broadcast_to([128, 4, d]),
            )
            nc.sync.dma_start(
                out=o_ap[:, quad * 4 : (quad + 1) * 4, n * d : (n + 1) * d],
                in_=ostage[:, quad * 4 : (quad + 1) * 4, :],
            )

        # software pipeline: pair p's eight mm2 half-units run during pair
        # p+1's steps 0..7 — every unit needs ALL of its head's Pt tiles
        # (all 8 j-blocks enter each accumulation).
        slot_at = {}
        for pair in range(heads // 2):
            for u in range(8):
                slot_at.setdefault((pair + 1) * nt + u, []).append((pair, u))
        states = []
        for pair in range(heads // 2):
            cur = load_and_transpose(pair)
            states.append(cur)
            for jb in range(nt):
                J = pair * nt + jb
                todo = slot_at.pop(J, [])
                # mm2 burst FIRST: PE work between exp(jb-1) and mm1(jb) so
                # the St tiles are free when the mm1 quartet issues.
                for p_, u in todo:
                    mm2_unit(states[p_], u)
                mm1_exp(cur, jb)
                if not todo and J < 7:
                    # keep the PE clock gate warm through the fill phase
                    pe_filler(2)
            mm1_tail(cur)
        for J in sorted(slot_at):
            for p_, u in slot_at[J]:
                mm2_unit(states[p_], u)

    # NOTE: a post-schedule PE-stream interleave pass was tried here (rebake
    # mm1 quartets adjacently); with the 3-deep St ring the exp latency chain
    # it addressed is gone, and reordering a wait-subsumption-optimized
    # stream is unsound without the full dep graph (races/deadlocks).
    _split_multi_waits(nc, mybir)
    return nc


def _get_nc():
    if "nc" not in _cache:
        _cache["nc"] = _build_nc()
    return _cache["nc"]


def _run(q, k, v, trace=False):
    from concourse.bass_utils import run_bass_kernel_spmd

    b, heads, h, w, d = 8, 8, 32, 32, 64
    q = np.ascontiguousarray(np.asarray(q, dtype=np.float32))
    k = np.ascontiguousarray(np.asarray(k, dtype=np.float32))
    v = np.ascontiguousarray(np.asarray(v, dtype=np.float32))
    assert q.shape == (b, heads, h, w, d), q.shape

    nc = _get_nc()
    in_maps = [
        {
            "q": q[c].reshape(heads, h * w, d),
            "k": k[c].reshape(heads, h * w, d),
            "v": v[c].reshape(heads, h * w, d),
        }
        for c in range(b)
    ]
    res = run_bass_kernel_spmd(nc, in_maps, core_ids=list(range(b)), trace=trace)
    out = np.stack(
        [res.results[c]["out"].reshape(h, w, heads * d) for c in range(b)]
    )
    return out, res


def kernel(q, k, v):
    out, _ = _run(q, k, v)
    return out

